# revision 2
# baseline (speedup 1.0000x reference)
"""Segment-mean (scatter_mean over sorted index) on Trainium2, 8 NeuronCores.

Strategy
--------
index is sorted, so segment s's edges are a contiguous row-range of x.
Output is processed in windows of P=128 segments. Window g (= core m,
local window w) covers segments [g*128, (g+1)*128) and draws from a
contiguous edge slice of x. The host pads every window's edge slice to a
fixed B*128 rows (B chosen from the data's max window population), which
makes the device program fully static and identical across the 8 cores —
all data-dependence lives in the per-core input tensors.

On device, per window:
  - load the window's [B*128, 128] x-slab as an SBUF tile [128, B*128]
    (partition = edge-within-tile, free = (tile, d))
  - for each of the B 128-edge tiles: build onehot[e, s] =
    (rel_idx[e] == s) via a DVE tensor_scalar is_equal against an iota
    tile, then matmul-accumulate psum[s, d] += onehot^T @ x_tile
  - multiply psum rows by 1/clamp(count,1) (per-partition scalar) and DMA
    the [128, 128] result to its static output rows.

Padding edges carry rel_idx = -1 -> all-zero onehot column -> no
contribution. Empty segments get sum 0 and count clamped to 1 -> output 0.
"""

import numpy as np

import concourse.bacc as bacc
import concourse.mybir as mybir
import concourse.tile as tile
from concourse.bass_utils import run_bass_kernel_spmd

P = 128
D = 128
NCORES = 8

_nc_cache: dict = {}


def _build(WN: int, B: int):
    """Compile the per-core SPMD program: WN windows, B edge-tiles each."""
    key = (WN, B)
    if key in _nc_cache:
        return _nc_cache[key]

    f32 = mybir.dt.float32
    T = WN * B  # edge tiles per core

    nc = bacc.Bacc("TRN2", target_bir_lowering=False, debug=False,
                   num_devices=NCORES)
    xg_d = nc.dram_tensor("xg", [T * P, D], f32, kind="ExternalInput").ap()
    rel_d = nc.dram_tensor("rel", [P, T], f32, kind="ExternalInput").ap()
    invc_d = nc.dram_tensor("invc", [P, WN], f32, kind="ExternalInput").ap()
    iota_d = nc.dram_tensor("iota", [P, P], f32, kind="ExternalInput").ap()
    out_d = nc.dram_tensor("out", [WN * P, D], f32, kind="ExternalOutput").ap()

    with tile.TileContext(nc) as tc:
        with (
            tc.tile_pool(name="const", bufs=1) as cpool,
            tc.tile_pool(name="xin", bufs=3) as xpool,
            tc.tile_pool(name="oh", bufs=12) as ohpool,
            tc.tile_pool(name="res", bufs=4) as rpool,
            tc.tile_pool(name="ps", bufs=4, space="PSUM") as pspool,
        ):
            iota_t = cpool.tile([P, P], f32)
            nc.sync.dma_start(out=iota_t[:], in_=iota_d[:])
            rel_t = cpool.tile([P, T], f32)
            nc.sync.dma_start(out=rel_t[:], in_=rel_d[:])
            invc_t = cpool.tile([P, WN], f32)
            nc.sync.dma_start(out=invc_t[:], in_=invc_d[:])

            # row (w*B + j)*P + p, col d  ->  [w][p][j][d]
            xg_r = xg_d.rearrange("(w j p) d -> w p j d", j=B, p=P)

            for w in range(WN):
                xw = xpool.tile([P, B * D], f32, tag="xw")
                nc.sync.dma_start(
                    out=xw[:].rearrange("p (j d) -> p j d", d=D),
                    in_=xg_r[w])
                ps = pspool.tile([P, D], f32, tag="ps")
                for j in range(B):
                    t = w * B + j
                    oh = ohpool.tile([P, P], f32, tag="oh")
                    nc.vector.tensor_scalar(
                        out=oh[:], in0=iota_t[:],
                        scalar1=rel_t[:, t:t + 1], scalar2=None,
                        op0=mybir.AluOpType.is_equal)
                    nc.tensor.matmul(
                        out=ps[:], lhsT=oh[:], rhs=xw[:, j * D:(j + 1) * D],
                        start=(j == 0), stop=(j == B - 1))
                res = rpool.tile([P, D], f32, tag="res")
                nc.vector.tensor_scalar_mul(
                    out=res[:], in0=ps[:], scalar1=invc_t[:, w:w + 1])
                nc.sync.dma_start(out=out_d[P * w:P * (w + 1), :], in_=res[:])

    nc.compile()
    _nc_cache[key] = nc
    return nc


def _prepare(x: np.ndarray, index: np.ndarray, n_segments: int):
    """Host-side shard/gather prep. Returns (WN, B, in_maps)."""
    E, d = x.shape
    assert d == D
    idx = np.asarray(index).astype(np.int64).ravel()

    if np.any(idx[1:] < idx[:-1]):  # tolerate unsorted input
        perm = np.argsort(idx, kind="stable")
        idx = idx[perm]
        x = x[perm]

    WN = -(-n_segments // (P * NCORES))     # windows per core
    G = WN * NCORES                         # total windows
    NPAD = G * P

    bounds = np.searchsorted(idx, np.arange(0, NPAD + 1, P)).astype(np.int64)
    wcnt = bounds[1:] - bounds[:-1]         # edges per window
    B = max(1, int(-(-int(wcnt.max()) // P)))  # edge tiles per window
    BP = B * P

    # Gather map: window g takes rows bounds[g] .. bounds[g]+BP-1 clamped
    # into the window (pads replicate the window's last valid row; fully
    # empty windows point at row 0 — masked out via rel=-1 either way).
    offs = np.tile(np.arange(BP, dtype=np.int64), G)
    base = np.repeat(bounds[:-1], BP)
    lim = np.repeat(np.maximum(wcnt, 1) - 1, BP)
    gi = base + np.minimum(offs, lim)
    np.clip(gi, 0, E - 1, out=gi)
    valid = offs < np.repeat(wcnt, BP)

    segbase = np.repeat(np.arange(G, dtype=np.int64) * P, BP)
    rel = np.where(valid, idx[gi] - segbase, -1).astype(np.float32)

    xg = x[gi]                              # [G*BP, D] float32
    cnt = np.bincount(idx, minlength=NPAD).astype(np.float32)
    inv = (1.0 / np.maximum(cnt, 1.0)).astype(np.float32)

    iota = np.ascontiguousarray(
        np.broadcast_to(np.arange(P, dtype=np.float32), (P, P)))

    xg = xg.reshape(NCORES, WN * BP, D)
    rel = rel.reshape(NCORES, WN * B, P)
    inv = inv.reshape(NCORES, WN, P)

    in_maps = []
    for m in range(NCORES):
        in_maps.append({
            "xg": np.ascontiguousarray(xg[m]),
            "rel": np.ascontiguousarray(rel[m].T),
            "invc": np.ascontiguousarray(inv[m].T),
            "iota": iota,
        })
    return WN, B, in_maps


def kernel_with_results(x, index, dim_size, **run_kwargs):
    x = np.ascontiguousarray(np.asarray(x, dtype=np.float32))
    n = int(np.asarray(dim_size))
    WN, B, in_maps = _prepare(x, np.asarray(index), n)
    nc = _build(WN, B)
    r = run_bass_kernel_spmd(nc, in_maps, core_ids=list(range(NCORES)),
                             **run_kwargs)
    out = np.concatenate([res["out"] for res in r.results], axis=0)[:n]
    return np.ascontiguousarray(out, dtype=np.float32), r


def kernel(x, index, dim_size):
    out, _ = kernel_with_results(x, index, dim_size)
    return out


# revision 4
# speedup vs baseline: 1.2020x; 1.2020x over previous
"""Segment-mean (scatter_mean over sorted index) on Trainium2, 8 NeuronCores.

Strategy
--------
index is sorted, so segment s's edges are a contiguous row-range of x.
The output is processed in windows of P=128 segments; window g draws from
a contiguous edge-slice of x. The host:
  * pre-scales every edge row by 1/clamp(count[seg],1)  (so segment-MEAN
    becomes plain segment-SUM on device),
  * splits the scaled rows into a bf16 hi/lo pair (hi = bf16(x),
    lo = bf16(x - hi); hi+lo carries ~17 mantissa bits, inside the fp32
    envelope, while enabling full-rate bf16 matmuls),
  * assigns windows to the 8 cores balanced by their edge-tile counts and
    pads each rank-w window to a common tile count B_w (max over cores),
    so the device program is fully static and identical across cores —
    all data-dependence lives in the per-core input tensors.

On device, per window (B = B_w edge tiles of 128 edges):
  * DMA the window's [B*128, 256] hi|lo slab into SBUF [128, B*256]
  * ONE batched DVE is_equal builds the bf16 one-hot [128, B*128]:
    onehot[e, j*128+s] = (rel[e, j] == s), using an iota tile and a
    step-0 broadcast AP of the per-tile relative indices
  * B matmuls accumulate psum[s, 0:128] += oh_j^T @ hi_j and
    psum[s, 128:256] += oh_j^T @ lo_j in one [K=128, N=256] pass each
  * one DVE add merges hi+lo sums out of PSUM -> SBUF -> DMA to the
    window's static output rows.

Padding edges carry rel = -1 -> all-zero one-hot column -> no
contribution. Empty segments have sum 0 -> output 0 (count clamped on
host).
"""

import numpy as np
import ml_dtypes

import concourse.bacc as bacc
import concourse.mybir as mybir
import concourse.tile as tile
from concourse.bass_utils import run_bass_kernel_spmd

P = 128
D = 128
NCORES = 8
BF16 = ml_dtypes.bfloat16

_nc_cache: dict = {}


def _build(Bs: tuple):
    """Compile the per-core SPMD program. Bs[w] = edge tiles in window w."""
    if Bs in _nc_cache:
        return _nc_cache[Bs]

    f32 = mybir.dt.float32
    bf16 = mybir.dt.bfloat16
    WN = len(Bs)
    T = int(sum(Bs))
    Bmax = int(max(Bs))
    offs = np.concatenate([[0], np.cumsum(Bs)]).astype(int)

    nc = bacc.Bacc("TRN2", target_bir_lowering=False, debug=False,
                   num_devices=NCORES)
    xhl_d = nc.dram_tensor("xhl", [T * P, 2 * D], bf16,
                           kind="ExternalInput").ap()
    rel_d = nc.dram_tensor("rel", [P, T], bf16, kind="ExternalInput").ap()
    iota_d = nc.dram_tensor("iota", [P, Bmax * P], bf16,
                            kind="ExternalInput").ap()
    out_d = nc.dram_tensor("out", [WN * P, D], f32, kind="ExternalOutput").ap()

    with tile.TileContext(nc) as tc:
        with (
            tc.tile_pool(name="const", bufs=1) as cpool,
            tc.tile_pool(name="xin", bufs=3) as xpool,
            tc.tile_pool(name="oh", bufs=4) as ohpool,
            tc.tile_pool(name="res", bufs=4) as rpool,
            tc.tile_pool(name="ps", bufs=4, space="PSUM") as pspool,
        ):
            iota_t = cpool.tile([P, Bmax * P], bf16)
            nc.sync.dma_start(out=iota_t[:], in_=iota_d[:])
            rel_t = cpool.tile([P, T], bf16)
            nc.sync.dma_start(out=rel_t[:], in_=rel_d[:])

            for w in range(WN):
                B = int(Bs[w])
                o = int(offs[w])
                xw = xpool.tile([P, Bmax * 2 * D], bf16, tag="xw")
                nc.sync.dma_start(
                    out=xw[:, :B * 2 * D].rearrange("p (j c) -> p j c",
                                                    c=2 * D),
                    in_=xhl_d[o * P:(o + B) * P, :].rearrange(
                        "(j p) c -> p j c", p=P))
                oh = ohpool.tile([P, Bmax * P], bf16, tag="oh")
                nc.vector.tensor_tensor(
                    out=oh[:, :B * P].rearrange("p (j s) -> p j s", s=P),
                    in0=iota_t[:, :B * P].rearrange("p (j s) -> p j s", s=P),
                    in1=rel_t[:, o:o + B].to_broadcast([P, B, P]),
                    op=mybir.AluOpType.is_equal)
                ps = pspool.tile([P, 2 * D], f32, tag="ps")
                for j in range(B):
                    nc.tensor.matmul(
                        out=ps[:],
                        lhsT=oh[:, j * P:(j + 1) * P],
                        rhs=xw[:, j * 2 * D:(j + 1) * 2 * D],
                        start=(j == 0), stop=(j == B - 1))
                res = rpool.tile([P, D], f32, tag="res")
                nc.vector.tensor_reduce(
                    out=res[:],
                    in_=ps[:].rearrange("p (k d) -> p d k", k=2),
                    axis=mybir.AxisListType.X, op=mybir.AluOpType.add)
                nc.sync.dma_start(out=out_d[P * w:P * (w + 1), :], in_=res[:])

    nc.compile()
    _nc_cache[Bs] = nc
    return nc


def _prepare(x: np.ndarray, index: np.ndarray, n_segments: int):
    """Host-side shard/gather prep.

    Returns (Bs, in_maps, asg) where asg[m][w] = global window id of core
    m's rank-w slot (or -1 for a dummy), for output reassembly.
    """
    E, d = x.shape
    assert d == D
    idx = np.asarray(index).astype(np.int64).ravel()

    if np.any(idx[1:] < idx[:-1]):  # tolerate unsorted input
        perm = np.argsort(idx, kind="stable")
        idx = idx[perm]
        x = x[perm]

    G = -(-n_segments // P)  # global 128-segment windows
    bounds = np.searchsorted(idx, np.arange(0, (G + 1) * P, P)).astype(np.int64)
    wcnt = bounds[1:] - bounds[:-1]
    wtiles = np.maximum(1, -(-wcnt // P))  # >=1 so every window is scheduled

    # Balance windows across cores by tile count (greedy, desc).
    order = np.argsort(-wtiles, kind="stable")
    loads = np.zeros(NCORES, np.int64)
    per_core: list[list[int]] = [[] for _ in range(NCORES)]
    for g in order:
        m = int(np.argmin(loads))
        per_core[m].append(int(g))
        loads[m] += wtiles[g]
    WN = max(len(c) for c in per_core)
    for m in range(NCORES):  # per-core desc by tiles (already desc by order)
        per_core[m] += [-1] * (WN - len(per_core[m]))
    asg = np.array(per_core)                          # [NCORES, WN]
    tl = np.where(asg >= 0, wtiles[np.maximum(asg, 0)], 1)
    Bs = tuple(int(b) for b in tl.max(axis=0))        # common schedule
    T = sum(Bs)
    offs = np.concatenate([[0], np.cumsum(Bs)]).astype(np.int64)

    # Pre-scale by 1/count and split to bf16 hi/lo.
    cnt = np.bincount(idx, minlength=n_segments).astype(np.float32)
    inv = (1.0 / np.maximum(cnt, 1.0)).astype(np.float32)
    xs = x * inv[idx][:, None]
    hi = xs.astype(BF16)
    lo = (xs - hi.astype(np.float32)).astype(BF16)

    in_maps = []
    iota = np.ascontiguousarray(np.broadcast_to(
        np.arange(max(Bs) * P, dtype=np.float32) % P,
        (P, max(Bs) * P))).astype(BF16)
    for m in range(NCORES):
        gi = np.zeros(T * P, np.int64)
        rel = np.full(T * P, -1.0, np.float32)
        for w in range(WN):
            g = asg[m, w]
            if g < 0:
                continue
            s0, c = bounds[g], int(wcnt[g])
            B = Bs[w]
            o = int(offs[w]) * P
            k = np.arange(B * P)
            rows = s0 + np.minimum(k, max(c - 1, 0))
            np.clip(rows, 0, E - 1, out=rows)
            gi[o:o + B * P] = rows
            valid = k < c
            rel[o:o + B * P] = np.where(valid, (idx[rows] - g * P), -1)
        xhl = np.empty((T * P, 2 * D), BF16)
        xhl[:, :D] = hi[gi]
        xhl[:, D:] = lo[gi]
        in_maps.append({
            "xhl": xhl,
            "rel": np.ascontiguousarray(rel.reshape(T, P).T.astype(BF16)),
            "iota": iota,
        })
    return Bs, in_maps, asg


def kernel_with_results(x, index, dim_size, **run_kwargs):
    x = np.ascontiguousarray(np.asarray(x, dtype=np.float32))
    n = int(np.asarray(dim_size))
    Bs, in_maps, asg = _prepare(x, np.asarray(index), n)
    nc = _build(Bs)
    r = run_bass_kernel_spmd(nc, in_maps, core_ids=list(range(NCORES)),
                             **run_kwargs)
    G = -(-n // P)
    out = np.zeros((G * P, D), np.float32)
    for m in range(NCORES):
        om = r.results[m]["out"]
        for w in range(asg.shape[1]):
            g = asg[m, w]
            if g >= 0:
                out[g * P:(g + 1) * P] = om[w * P:(w + 1) * P]
    return np.ascontiguousarray(out[:n]), r


def kernel(x, index, dim_size):
    out, _ = kernel_with_results(x, index, dim_size)
    return out


# revision 5
# speedup vs baseline: 1.2601x; 1.0483x over previous
"""Segment-mean (scatter_mean over sorted index) on Trainium2, 8 NeuronCores.

Strategy
--------
index is sorted, so segment s's edges are a contiguous row-range of x.
The output is processed in windows of P=128 segments; window g draws from
a contiguous edge-slice of x. The host:
  * pre-scales every edge row by 1/clamp(count[seg],1)  (so segment-MEAN
    becomes plain segment-SUM on device),
  * splits the scaled rows into a bf16 hi/lo pair (hi = bf16(x),
    lo = bf16(x - hi); hi+lo carries ~17 mantissa bits, inside the fp32
    envelope, while enabling full-rate bf16 matmuls),
  * assigns windows to the 8 cores balanced by their edge-tile counts and
    pads each rank-w window to a common tile count B_w (max over cores),
    so the device program is fully static and identical across cores —
    all data-dependence lives in the per-core input tensors.

On device, per window (B = B_w edge tiles of 128 edges):
  * DMA the window's [B*128, 256] hi|lo slab into SBUF [128, B*256]
  * ONE batched DVE is_equal builds the bf16 one-hot [128, B*128]:
    onehot[e, j*128+s] = (rel[e, j] == s), using an iota tile and a
    step-0 broadcast AP of the per-tile relative indices
  * B matmuls accumulate psum[s, 0:128] += oh_j^T @ hi_j and
    psum[s, 128:256] += oh_j^T @ lo_j in one [K=128, N=256] pass each
  * one DVE add merges hi+lo sums out of PSUM -> SBUF -> DMA to the
    window's static output rows.

Padding edges carry rel = -1 -> all-zero one-hot column -> no
contribution. Empty segments have sum 0 -> output 0 (count clamped on
host).
"""

import numpy as np
import ml_dtypes

import concourse.bacc as bacc
import concourse.mybir as mybir
import concourse.tile as tile
from concourse.bass_utils import run_bass_kernel_spmd

P = 128
D = 128
NCORES = 8
BF16 = ml_dtypes.bfloat16

_nc_cache: dict = {}


def _build(Bs: tuple):
    """Compile the per-core SPMD program. Bs[w] = edge tiles in window w."""
    if Bs in _nc_cache:
        return _nc_cache[Bs]

    f32 = mybir.dt.float32
    bf16 = mybir.dt.bfloat16
    WN = len(Bs)
    T = int(sum(Bs))
    Bmax = int(max(Bs))
    offs = np.concatenate([[0], np.cumsum(Bs)]).astype(int)

    nc = bacc.Bacc("TRN2", target_bir_lowering=False, debug=False,
                   num_devices=NCORES)
    xhl_d = nc.dram_tensor("xhl", [T * P, 2 * D], bf16,
                           kind="ExternalInput").ap()
    rel_d = nc.dram_tensor("rel", [P, T], bf16, kind="ExternalInput").ap()
    iota_d = nc.dram_tensor("iota", [P, Bmax * P], bf16,
                            kind="ExternalInput").ap()
    out_d = nc.dram_tensor("out", [WN * P, D], f32, kind="ExternalOutput").ap()

    with tile.TileContext(nc) as tc:
        with (
            tc.tile_pool(name="const", bufs=1) as cpool,
            tc.tile_pool(name="xin", bufs=3) as xpool,
            tc.tile_pool(name="oh", bufs=4) as ohpool,
            tc.tile_pool(name="pair", bufs=3) as ppool,
            tc.tile_pool(name="res", bufs=3) as rpool,
            tc.tile_pool(name="ps", bufs=4, space="PSUM") as pspool,
        ):
            iota_t = cpool.tile([P, Bmax * P], bf16)
            nc.sync.dma_start(out=iota_t[:], in_=iota_d[:])
            rel_t = cpool.tile([P, T], bf16)
            nc.sync.dma_start(out=rel_t[:], in_=rel_d[:])

            for w in range(WN):
                B = int(Bs[w])
                o = int(offs[w])
                xw = xpool.tile([P, Bmax * 2 * D], bf16, tag="xw")
                nc.sync.dma_start(
                    out=xw[:, :B * 2 * D].rearrange("p (j c) -> p j c",
                                                    c=2 * D),
                    in_=xhl_d[o * P:(o + B) * P, :].rearrange(
                        "(j p) c -> p j c", p=P))
                oh = ohpool.tile([P, Bmax * P], bf16, tag="oh")
                nc.vector.tensor_tensor(
                    out=oh[:, :B * P].rearrange("p (j s) -> p j s", s=P),
                    in0=iota_t[:, :B * P].rearrange("p (j s) -> p j s", s=P),
                    in1=rel_t[:, o:o + B].to_broadcast([P, B, P]),
                    op=mybir.AluOpType.is_equal)
                ps = pspool.tile([P, 2 * D], f32, tag="ps")
                for j in range(B):
                    nc.tensor.matmul(
                        out=ps[:],
                        lhsT=oh[:, j * P:(j + 1) * P],
                        rhs=xw[:, j * 2 * D:(j + 1) * 2 * D],
                        start=(j == 0), stop=(j == B - 1))
                # extraction off the DVE: ACT copies the hi|lo pair out of
                # PSUM, GpSimd adds the halves, store triggers on ACT's DGE
                pair = ppool.tile([P, 2 * D], f32, tag="pair")
                nc.scalar.copy(out=pair[:], in_=ps[:])
                res = rpool.tile([P, D], f32, tag="res")
                nc.gpsimd.tensor_tensor(out=res[:], in0=pair[:, :D],
                                        in1=pair[:, D:2 * D],
                                        op=mybir.AluOpType.add)
                nc.scalar.dma_start(out=out_d[P * w:P * (w + 1), :],
                                    in_=res[:])

    nc.compile()
    _nc_cache[Bs] = nc
    return nc


def _prepare(x: np.ndarray, index: np.ndarray, n_segments: int):
    """Host-side shard/gather prep.

    Returns (Bs, in_maps, asg) where asg[m][w] = global window id of core
    m's rank-w slot (or -1 for a dummy), for output reassembly.
    """
    E, d = x.shape
    assert d == D
    idx = np.asarray(index).astype(np.int64).ravel()

    if np.any(idx[1:] < idx[:-1]):  # tolerate unsorted input
        perm = np.argsort(idx, kind="stable")
        idx = idx[perm]
        x = x[perm]

    G = -(-n_segments // P)  # global 128-segment windows
    bounds = np.searchsorted(idx, np.arange(0, (G + 1) * P, P)).astype(np.int64)
    wcnt = bounds[1:] - bounds[:-1]
    wtiles = np.maximum(1, -(-wcnt // P))  # >=1 so every window is scheduled

    # Balance windows across cores by tile count (greedy, desc).
    order = np.argsort(-wtiles, kind="stable")
    loads = np.zeros(NCORES, np.int64)
    per_core: list[list[int]] = [[] for _ in range(NCORES)]
    for g in order:
        m = int(np.argmin(loads))
        per_core[m].append(int(g))
        loads[m] += wtiles[g]
    WN = max(len(c) for c in per_core)
    for m in range(NCORES):  # per-core desc by tiles (already desc by order)
        per_core[m] += [-1] * (WN - len(per_core[m]))
    asg = np.array(per_core)                          # [NCORES, WN]
    tl = np.where(asg >= 0, wtiles[np.maximum(asg, 0)], 1)
    Bs = tuple(int(b) for b in tl.max(axis=0))        # common schedule
    T = sum(Bs)
    offs = np.concatenate([[0], np.cumsum(Bs)]).astype(np.int64)

    # Pre-scale by 1/count and split to bf16 hi/lo.
    cnt = np.bincount(idx, minlength=n_segments).astype(np.float32)
    inv = (1.0 / np.maximum(cnt, 1.0)).astype(np.float32)
    xs = x * inv[idx][:, None]
    hi = xs.astype(BF16)
    lo = (xs - hi.astype(np.float32)).astype(BF16)

    in_maps = []
    iota = np.ascontiguousarray(np.broadcast_to(
        np.arange(max(Bs) * P, dtype=np.float32) % P,
        (P, max(Bs) * P))).astype(BF16)
    for m in range(NCORES):
        gi = np.zeros(T * P, np.int64)
        rel = np.full(T * P, -1.0, np.float32)
        for w in range(WN):
            g = asg[m, w]
            if g < 0:
                continue
            s0, c = bounds[g], int(wcnt[g])
            B = Bs[w]
            o = int(offs[w]) * P
            k = np.arange(B * P)
            rows = s0 + np.minimum(k, max(c - 1, 0))
            np.clip(rows, 0, E - 1, out=rows)
            gi[o:o + B * P] = rows
            valid = k < c
            rel[o:o + B * P] = np.where(valid, (idx[rows] - g * P), -1)
        xhl = np.empty((T * P, 2 * D), BF16)
        xhl[:, :D] = hi[gi]
        xhl[:, D:] = lo[gi]
        in_maps.append({
            "xhl": xhl,
            "rel": np.ascontiguousarray(rel.reshape(T, P).T.astype(BF16)),
            "iota": iota,
        })
    return Bs, in_maps, asg


def kernel_with_results(x, index, dim_size, **run_kwargs):
    x = np.ascontiguousarray(np.asarray(x, dtype=np.float32))
    n = int(np.asarray(dim_size))
    Bs, in_maps, asg = _prepare(x, np.asarray(index), n)
    nc = _build(Bs)
    r = run_bass_kernel_spmd(nc, in_maps, core_ids=list(range(NCORES)),
                             **run_kwargs)
    G = -(-n // P)
    out = np.zeros((G * P, D), np.float32)
    for m in range(NCORES):
        om = r.results[m]["out"]
        for w in range(asg.shape[1]):
            g = asg[m, w]
            if g >= 0:
                out[g * P:(g + 1) * P] = om[w * P:(w + 1) * P]
    return np.ascontiguousarray(out[:n]), r


def kernel(x, index, dim_size):
    out, _ = kernel_with_results(x, index, dim_size)
    return out


# revision 6
# speedup vs baseline: 1.4145x; 1.1226x over previous
"""Segment-mean (scatter_mean over sorted index) on Trainium2, 8 NeuronCores.

Strategy
--------
index is sorted, so segment s's edges are a contiguous row-range of x.
The output is processed in windows of P=128 segments; window g draws from
a contiguous edge-slice of x. The host:
  * pre-scales every edge row by 1/clamp(count[seg],1)  (so segment-MEAN
    becomes plain segment-SUM on device),
  * splits the scaled rows into a bf16 hi/lo pair (hi = bf16(x),
    lo = bf16(x - hi); hi+lo carries ~17 mantissa bits, inside the fp32
    envelope, while enabling full-rate bf16 matmuls),
  * assigns windows to the 8 cores balanced by their edge-tile counts and
    pads each rank-w window to a common tile count B_w (max over cores),
    so the device program is fully static and identical across cores —
    all data-dependence lives in the per-core input tensors.

On device, per window (B = B_w edge tiles of 128 edges):
  * DMA the window's [B*128, 256] hi|lo slab into SBUF [128, B*256]
  * ONE batched DVE is_equal builds the bf16 one-hot [128, B*128]:
    onehot[e, j*128+s] = (rel[e, j] == s), using an iota tile and a
    step-0 broadcast AP of the per-tile relative indices
  * B matmuls accumulate psum[s, 0:128] += oh_j^T @ hi_j and
    psum[s, 128:256] += oh_j^T @ lo_j in one [K=128, N=256] pass each
  * one DVE add merges hi+lo sums out of PSUM -> SBUF -> DMA to the
    window's static output rows.

Padding edges carry rel = -1 -> all-zero one-hot column -> no
contribution. Empty segments have sum 0 -> output 0 (count clamped on
host).
"""

import numpy as np
import ml_dtypes

import concourse.bacc as bacc
import concourse.mybir as mybir
import concourse.tile as tile
from concourse.bass_utils import run_bass_kernel_spmd

P = 128
D = 128
NCORES = 8
BF16 = ml_dtypes.bfloat16

_nc_cache: dict = {}


def _build(Bs: tuple):
    """Compile the per-core SPMD program. Bs[w] = edge tiles in window w."""
    if Bs in _nc_cache:
        return _nc_cache[Bs]

    f32 = mybir.dt.float32
    bf16 = mybir.dt.bfloat16
    WN = len(Bs)
    T = int(sum(Bs))
    Bmax = int(max(Bs))
    offs = np.concatenate([[0], np.cumsum(Bs)]).astype(int)

    nc = bacc.Bacc("TRN2", target_bir_lowering=False, debug=False,
                   num_devices=NCORES)
    xhl_d = nc.dram_tensor("xhl", [T * P, 2 * D], bf16,
                           kind="ExternalInput").ap()
    rel_d = nc.dram_tensor("rel", [P, T], bf16, kind="ExternalInput").ap()
    iota_d = nc.dram_tensor("iota", [P, Bmax * P], bf16,
                            kind="ExternalInput").ap()
    out_d = nc.dram_tensor("out", [WN * P, D], f32, kind="ExternalOutput").ap()

    # window pairs share one ~1MB load
    pairs = [(w, min(w + 1, WN - 1)) if w + 1 < WN else (w, None)
             for w in range(0, WN, 2)]

    with tile.TileContext(nc) as tc:
        with (
            tc.tile_pool(name="const", bufs=1) as cpool,
            tc.tile_pool(name="xin", bufs=4) as xpool,
            tc.tile_pool(name="oh", bufs=8) as ohpool,
            tc.tile_pool(name="pair", bufs=4) as ppool,
            tc.tile_pool(name="res", bufs=4) as rpool,
            tc.tile_pool(name="ps", bufs=6, space="PSUM") as pspool,
            tc.tile_pool(name="pswarm", bufs=1, space="PSUM") as wpool,
        ):
            iota_t = cpool.tile([P, Bmax * P], bf16)
            nc.sync.dma_start(out=iota_t[:], in_=iota_d[:])
            rel_t = cpool.tile([P, T], bf16)
            nc.sync.dma_start(out=rel_t[:], in_=rel_d[:])

            # ~5us of dummy matmuls to flip the PE HAM clock-gate to 8/8
            # before the first real accumulation group arrives.
            warm = wpool.tile([P, 2 * D], f32)
            for _ in range(24):
                nc.tensor.matmul(out=warm[:], lhsT=iota_t[:, :P],
                                 rhs=iota_t[:, :2 * D], start=True, stop=True)

            for wa, wb in pairs:
                Ba = int(Bs[wa])
                Bb = int(Bs[wb]) if wb is not None else 0
                o = int(offs[wa])
                Bt = Ba + Bb
                xw = xpool.tile([P, 2 * Bmax * 2 * D], bf16, tag="xw")
                nc.sync.dma_start(
                    out=xw[:, :Bt * 2 * D].rearrange("p (j c) -> p j c",
                                                     c=2 * D),
                    in_=xhl_d[o * P:(o + Bt) * P, :].rearrange(
                        "(j p) c -> p j c", p=P))
                for w, B, jo in (((wa, Ba, 0),) if wb is None else
                                 ((wa, Ba, 0), (wb, Bb, Ba))):
                    ow = int(offs[w])
                    oh = ohpool.tile([P, Bmax * P], bf16, tag="oh")
                    nc.vector.tensor_tensor(
                        out=oh[:, :B * P].rearrange("p (j s) -> p j s", s=P),
                        in0=iota_t[:, :B * P].rearrange("p (j s) -> p j s",
                                                        s=P),
                        in1=rel_t[:, ow:ow + B].to_broadcast([P, B, P]),
                        op=mybir.AluOpType.is_equal)
                    ps = pspool.tile([P, 2 * D], f32, tag="ps")
                    for j in range(B):
                        nc.tensor.matmul(
                            out=ps[:],
                            lhsT=oh[:, j * P:(j + 1) * P],
                            rhs=xw[:, (jo + j) * 2 * D:(jo + j + 1) * 2 * D],
                            start=(j == 0), stop=(j == B - 1))
                    # extraction off the DVE: ACT copies the hi|lo pair out
                    # of PSUM, GpSimd adds the halves, store on ACT's DGE
                    pair = ppool.tile([P, 2 * D], f32, tag="pair")
                    nc.scalar.copy(out=pair[:], in_=ps[:])
                    res = rpool.tile([P, D], f32, tag="res")
                    nc.gpsimd.tensor_tensor(out=res[:], in0=pair[:, :D],
                                            in1=pair[:, D:2 * D],
                                            op=mybir.AluOpType.add)
                    nc.scalar.dma_start(out=out_d[P * w:P * (w + 1), :],
                                        in_=res[:])

    nc.compile()
    _nc_cache[Bs] = nc
    return nc


def _prepare(x: np.ndarray, index: np.ndarray, n_segments: int):
    """Host-side shard/gather prep.

    Returns (Bs, in_maps, asg) where asg[m][w] = global window id of core
    m's rank-w slot (or -1 for a dummy), for output reassembly.
    """
    E, d = x.shape
    assert d == D
    idx = np.asarray(index).astype(np.int64).ravel()

    if np.any(idx[1:] < idx[:-1]):  # tolerate unsorted input
        perm = np.argsort(idx, kind="stable")
        idx = idx[perm]
        x = x[perm]

    G = -(-n_segments // P)  # global 128-segment windows
    bounds = np.searchsorted(idx, np.arange(0, (G + 1) * P, P)).astype(np.int64)
    wcnt = bounds[1:] - bounds[:-1]
    wtiles = np.maximum(1, -(-wcnt // P))  # >=1 so every window is scheduled

    # Balance windows across cores by tile count (greedy, desc).
    order = np.argsort(-wtiles, kind="stable")
    loads = np.zeros(NCORES, np.int64)
    per_core: list[list[int]] = [[] for _ in range(NCORES)]
    for g in order:
        m = int(np.argmin(loads))
        per_core[m].append(int(g))
        loads[m] += wtiles[g]
    WN = max(len(c) for c in per_core)
    for m in range(NCORES):  # per-core desc by tiles (already desc by order)
        per_core[m] += [-1] * (WN - len(per_core[m]))
    asg = np.array(per_core)                          # [NCORES, WN]
    tl = np.where(asg >= 0, wtiles[np.maximum(asg, 0)], 1)
    Bs = tuple(int(b) for b in tl.max(axis=0))        # common schedule
    T = sum(Bs)
    offs = np.concatenate([[0], np.cumsum(Bs)]).astype(np.int64)

    # Pre-scale by 1/count and split to bf16 hi/lo.
    cnt = np.bincount(idx, minlength=n_segments).astype(np.float32)
    inv = (1.0 / np.maximum(cnt, 1.0)).astype(np.float32)
    xs = x * inv[idx][:, None]
    hi = xs.astype(BF16)
    lo = (xs - hi.astype(np.float32)).astype(BF16)

    in_maps = []
    iota = np.ascontiguousarray(np.broadcast_to(
        np.arange(max(Bs) * P, dtype=np.float32) % P,
        (P, max(Bs) * P))).astype(BF16)
    for m in range(NCORES):
        gi = np.zeros(T * P, np.int64)
        rel = np.full(T * P, -1.0, np.float32)
        for w in range(WN):
            g = asg[m, w]
            if g < 0:
                continue
            s0, c = bounds[g], int(wcnt[g])
            B = Bs[w]
            o = int(offs[w]) * P
            k = np.arange(B * P)
            rows = s0 + np.minimum(k, max(c - 1, 0))
            np.clip(rows, 0, E - 1, out=rows)
            gi[o:o + B * P] = rows
            valid = k < c
            rel[o:o + B * P] = np.where(valid, (idx[rows] - g * P), -1)
        xhl = np.empty((T * P, 2 * D), BF16)
        xhl[:, :D] = hi[gi]
        xhl[:, D:] = lo[gi]
        in_maps.append({
            "xhl": xhl,
            "rel": np.ascontiguousarray(rel.reshape(T, P).T.astype(BF16)),
            "iota": iota,
        })
    return Bs, in_maps, asg


def kernel_with_results(x, index, dim_size, **run_kwargs):
    x = np.ascontiguousarray(np.asarray(x, dtype=np.float32))
    n = int(np.asarray(dim_size))
    Bs, in_maps, asg = _prepare(x, np.asarray(index), n)
    nc = _build(Bs)
    r = run_bass_kernel_spmd(nc, in_maps, core_ids=list(range(NCORES)),
                             **run_kwargs)
    G = -(-n // P)
    out = np.zeros((G * P, D), np.float32)
    for m in range(NCORES):
        om = r.results[m]["out"]
        for w in range(asg.shape[1]):
            g = asg[m, w]
            if g >= 0:
                out[g * P:(g + 1) * P] = om[w * P:(w + 1) * P]
    return np.ascontiguousarray(out[:n]), r


def kernel(x, index, dim_size):
    out, _ = kernel_with_results(x, index, dim_size)
    return out


# revision 7
# speedup vs baseline: 1.5168x; 1.0723x over previous
"""Segment-mean (scatter_mean over sorted index) on Trainium2, 8 NeuronCores.

Strategy
--------
index is sorted, so segment s's edges are a contiguous row-range of x.
The output is processed in windows of P=128 segments; window g draws from
a contiguous edge-slice of x. The host:
  * pre-scales every edge row by 1/clamp(count[seg],1)  (so segment-MEAN
    becomes plain segment-SUM on device),
  * splits the scaled rows into a bf16 hi/lo pair (hi = bf16(x),
    lo = bf16(x - hi); hi+lo carries ~17 mantissa bits, inside the fp32
    envelope, while enabling full-rate bf16 matmuls),
  * assigns windows to the 8 cores balanced by their edge-tile counts and
    pads each rank-w window to a common tile count B_w (max over cores),
    so the device program is fully static and identical across cores —
    all data-dependence lives in the per-core input tensors.

On device, per window (B = B_w edge tiles of 128 edges):
  * DMA the window's [B*128, 256] hi|lo slab into SBUF [128, B*256]
  * ONE batched DVE is_equal builds the bf16 one-hot [128, B*128]:
    onehot[e, j*128+s] = (rel[e, j] == s), using an iota tile and a
    step-0 broadcast AP of the per-tile relative indices
  * B matmuls accumulate psum[s, 0:128] += oh_j^T @ hi_j and
    psum[s, 128:256] += oh_j^T @ lo_j in one [K=128, N=256] pass each
  * one DVE add merges hi+lo sums out of PSUM -> SBUF -> DMA to the
    window's static output rows.

Padding edges carry rel = -1 -> all-zero one-hot column -> no
contribution. Empty segments have sum 0 -> output 0 (count clamped on
host).
"""

import numpy as np
import ml_dtypes

import concourse.bacc as bacc
import concourse.mybir as mybir
import concourse.tile as tile
from concourse.bass_utils import run_bass_kernel_spmd

P = 128
D = 128
NCORES = 8
BF16 = ml_dtypes.bfloat16

_nc_cache: dict = {}


def _build(Bs: tuple):
    """Compile the per-core SPMD program. Bs[w] = edge tiles in window w."""
    if Bs in _nc_cache:
        return _nc_cache[Bs]

    f32 = mybir.dt.float32
    bf16 = mybir.dt.bfloat16
    WN = len(Bs)
    T = int(sum(Bs))
    Bmax = int(max(Bs))
    offs = np.concatenate([[0], np.cumsum(Bs)]).astype(int)

    nc = bacc.Bacc("TRN2", target_bir_lowering=False, debug=False,
                   num_devices=NCORES)
    xhl_d = nc.dram_tensor("xhl", [T * P, 2 * D], bf16,
                           kind="ExternalInput").ap()
    rel_d = nc.dram_tensor("rel", [P, T], bf16, kind="ExternalInput").ap()
    iota_d = nc.dram_tensor("iota", [P, Bmax * P], bf16,
                            kind="ExternalInput").ap()
    out_d = nc.dram_tensor("out", [WN * P, D], f32, kind="ExternalOutput").ap()

    # window pairs share one ~1MB load
    pairs = [(w, min(w + 1, WN - 1)) if w + 1 < WN else (w, None)
             for w in range(0, WN, 2)]

    with tile.TileContext(nc) as tc:
        with (
            tc.tile_pool(name="const", bufs=1) as cpool,
            tc.tile_pool(name="xin", bufs=6) as xpool,
            tc.tile_pool(name="oh", bufs=8) as ohpool,
            tc.tile_pool(name="res", bufs=6) as rpool,
            tc.tile_pool(name="ps", bufs=6, space="PSUM") as pspool,
            tc.tile_pool(name="pswarm", bufs=1, space="PSUM") as wpool,
        ):
            iota_t = cpool.tile([P, Bmax * P], bf16)
            nc.sync.dma_start(out=iota_t[:], in_=iota_d[:])
            rel_t = cpool.tile([P, T], bf16)
            nc.sync.dma_start(out=rel_t[:], in_=rel_d[:])

            # ~5us of dummy matmuls to flip the PE HAM clock-gate to 8/8
            # before the first real accumulation group arrives.
            warm = wpool.tile([P, 2 * D], f32)
            for _ in range(24):
                nc.tensor.matmul(out=warm[:], lhsT=iota_t[:, :P],
                                 rhs=iota_t[:, :2 * D], start=True, stop=True)

            for wa, wb in pairs:
                Ba = int(Bs[wa])
                Bb = int(Bs[wb]) if wb is not None else 0
                o = int(offs[wa])
                Bt = Ba + Bb
                xw = xpool.tile([P, 2 * Bmax * 2 * D], bf16, tag="xw")
                nc.sync.dma_start(
                    out=xw[:, :Bt * 2 * D].rearrange("p (j c) -> p j c",
                                                     c=2 * D),
                    in_=xhl_d[o * P:(o + Bt) * P, :].rearrange(
                        "(j p) c -> p j c", p=P))
                nw = 1 if wb is None else 2
                res = rpool.tile([P, nw * D], f32, tag="res")
                for w, B, jo, ri in (((wa, Ba, 0, 0),) if wb is None else
                                     ((wa, Ba, 0, 0), (wb, Bb, Ba, 1))):
                    ow = int(offs[w])
                    oh = ohpool.tile([P, Bmax * P], bf16, tag="oh")
                    nc.vector.tensor_tensor(
                        out=oh[:, :B * P].rearrange("p (j s) -> p j s", s=P),
                        in0=iota_t[:, :B * P].rearrange("p (j s) -> p j s",
                                                        s=P),
                        in1=rel_t[:, ow:ow + B].to_broadcast([P, B, P]),
                        op=mybir.AluOpType.is_equal)
                    ps = pspool.tile([P, D], f32, tag="ps")
                    for j in range(B):
                        c0 = (jo + j) * 2 * D
                        nc.tensor.matmul(out=ps[:],
                                         lhsT=oh[:, j * P:(j + 1) * P],
                                         rhs=xw[:, c0:c0 + D],
                                         start=(j == 0), stop=False)
                        nc.tensor.matmul(out=ps[:],
                                         lhsT=oh[:, j * P:(j + 1) * P],
                                         rhs=xw[:, c0 + D:c0 + 2 * D],
                                         start=False, stop=(j == B - 1))
                    # single cheap extraction on the (idle) scalar engine
                    nc.scalar.copy(out=res[:, ri * D:(ri + 1) * D], in_=ps[:])
                # one store per window pair, on ACT's DGE
                nc.scalar.dma_start(
                    out=out_d[P * wa:P * (wa + nw), :].rearrange(
                        "(k r) d -> r k d", r=P),
                    in_=res[:].rearrange("p (k d) -> p k d", d=D))

    nc.compile()
    _nc_cache[Bs] = nc
    return nc


def _prepare(x: np.ndarray, index: np.ndarray, n_segments: int):
    """Host-side shard/gather prep.

    Returns (Bs, in_maps, asg) where asg[m][w] = global window id of core
    m's rank-w slot (or -1 for a dummy), for output reassembly.
    """
    E, d = x.shape
    assert d == D
    idx = np.asarray(index).astype(np.int64).ravel()

    if np.any(idx[1:] < idx[:-1]):  # tolerate unsorted input
        perm = np.argsort(idx, kind="stable")
        idx = idx[perm]
        x = x[perm]

    G = -(-n_segments // P)  # global 128-segment windows
    bounds = np.searchsorted(idx, np.arange(0, (G + 1) * P, P)).astype(np.int64)
    wcnt = bounds[1:] - bounds[:-1]
    wtiles = np.maximum(1, -(-wcnt // P))  # >=1 so every window is scheduled

    # Balance windows across cores by tile count (greedy, desc).
    order = np.argsort(-wtiles, kind="stable")
    loads = np.zeros(NCORES, np.int64)
    per_core: list[list[int]] = [[] for _ in range(NCORES)]
    for g in order:
        m = int(np.argmin(loads))
        per_core[m].append(int(g))
        loads[m] += wtiles[g]
    WN = max(len(c) for c in per_core)
    for m in range(NCORES):  # per-core desc by tiles (already desc by order)
        per_core[m] += [-1] * (WN - len(per_core[m]))
    asg = np.array(per_core)                          # [NCORES, WN]
    tl = np.where(asg >= 0, wtiles[np.maximum(asg, 0)], 1)
    Bs = tuple(int(b) for b in tl.max(axis=0))        # common schedule
    T = sum(Bs)
    offs = np.concatenate([[0], np.cumsum(Bs)]).astype(np.int64)

    # Pre-scale by 1/count and split to bf16 hi/lo.
    cnt = np.bincount(idx, minlength=n_segments).astype(np.float32)
    inv = (1.0 / np.maximum(cnt, 1.0)).astype(np.float32)
    xs = x * inv[idx][:, None]
    hi = xs.astype(BF16)
    lo = (xs - hi.astype(np.float32)).astype(BF16)

    in_maps = []
    iota = np.ascontiguousarray(np.broadcast_to(
        np.arange(max(Bs) * P, dtype=np.float32) % P,
        (P, max(Bs) * P))).astype(BF16)
    for m in range(NCORES):
        gi = np.zeros(T * P, np.int64)
        rel = np.full(T * P, -1.0, np.float32)
        for w in range(WN):
            g = asg[m, w]
            if g < 0:
                continue
            s0, c = bounds[g], int(wcnt[g])
            B = Bs[w]
            o = int(offs[w]) * P
            k = np.arange(B * P)
            rows = s0 + np.minimum(k, max(c - 1, 0))
            np.clip(rows, 0, E - 1, out=rows)
            gi[o:o + B * P] = rows
            valid = k < c
            rel[o:o + B * P] = np.where(valid, (idx[rows] - g * P), -1)
        xhl = np.empty((T * P, 2 * D), BF16)
        xhl[:, :D] = hi[gi]
        xhl[:, D:] = lo[gi]
        in_maps.append({
            "xhl": xhl,
            "rel": np.ascontiguousarray(rel.reshape(T, P).T.astype(BF16)),
            "iota": iota,
        })
    return Bs, in_maps, asg


def kernel_with_results(x, index, dim_size, **run_kwargs):
    x = np.ascontiguousarray(np.asarray(x, dtype=np.float32))
    n = int(np.asarray(dim_size))
    Bs, in_maps, asg = _prepare(x, np.asarray(index), n)
    nc = _build(Bs)
    r = run_bass_kernel_spmd(nc, in_maps, core_ids=list(range(NCORES)),
                             **run_kwargs)
    G = -(-n // P)
    out = np.zeros((G * P, D), np.float32)
    for m in range(NCORES):
        om = r.results[m]["out"]
        for w in range(asg.shape[1]):
            g = asg[m, w]
            if g >= 0:
                out[g * P:(g + 1) * P] = om[w * P:(w + 1) * P]
    return np.ascontiguousarray(out[:n]), r


def kernel(x, index, dim_size):
    out, _ = kernel_with_results(x, index, dim_size)
    return out
